# revision 1
# baseline (speedup 1.0000x reference)
"""Trainium2 Bass kernel for nn_BMSampling: out = X.reshape(B*C, T) @ smp_weight.

Strategy: tensor-parallel over the N*D*T = 320000 output columns, 8 cores x
40000 columns each. Each core runs the same program on its column shard:
  OUT_shard[512, 40000] = X^T[100, 512].T @ W_shard[100, 40000]
The kernel is output-DMA bound (~82 MB written per core), so the loop
structure streams W tiles in, keeps X^T resident in SBUF as the stationary
matmul operand, and stages 2 MB output tiles for efficient DMA out.
"""

from contextlib import ExitStack

import numpy as np

import concourse.bacc as bacc
import concourse.mybir as mybir
import concourse.tile as tile
from concourse import bass_utils

B, C, T = 4, 128, 100
N_SMP, D_PROP = 32, 100
M = B * C                     # 512 matmul rows
NDT = N_SMP * D_PROP * T      # 320000 output columns
NCORES = 8
NSH = NDT // NCORES           # 40000 columns per core

K = T                         # 100 contraction dim (on SBUF partitions)
N_OUTER = 4000                # columns per W tile / output staging tile
N_INNER = 500                 # matmul free dim (fits one PSUM bank: <=512 f32)
F32 = mybir.dt.float32

_PROGRAM = None


def _build():
    global _PROGRAM
    if _PROGRAM is not None:
        return _PROGRAM

    nc = bacc.Bacc("TRN2", debug=False)
    xt = nc.dram_tensor("XT", [K, M], F32, kind="ExternalInput").ap()
    w = nc.dram_tensor("W", [K, NSH], F32, kind="ExternalInput").ap()
    out = nc.dram_tensor("OUT", [M, NSH], F32, kind="ExternalOutput").ap()

    with tile.TileContext(nc) as tc, ExitStack() as ctx:
        xpool = ctx.enter_context(tc.tile_pool(name="x", bufs=1))
        wpool = ctx.enter_context(tc.tile_pool(name="w", bufs=3))
        opool = ctx.enter_context(tc.tile_pool(name="o", bufs=3))
        pspool = ctx.enter_context(tc.tile_pool(name="ps", bufs=8, space="PSUM"))

        xt_sb = xpool.tile([K, M], F32)
        nc.sync.dma_start(out=xt_sb[:], in_=xt[:])

        for n0 in range(0, NSH, N_OUTER):
            w_sb = wpool.tile([K, N_OUTER], F32)
            nc.sync.dma_start(out=w_sb[:], in_=w[:, n0 : n0 + N_OUTER])
            for m in range(M // 128):
                o_sb = opool.tile([128, N_OUTER], F32)
                for j in range(N_OUTER // N_INNER):
                    ps = pspool.tile([128, N_INNER], F32)
                    nc.tensor.matmul(
                        ps[:],
                        xt_sb[:, m * 128 : (m + 1) * 128],
                        w_sb[:, j * N_INNER : (j + 1) * N_INNER],
                        start=True,
                        stop=True,
                    )
                    nc.vector.tensor_copy(
                        out=o_sb[:, j * N_INNER : (j + 1) * N_INNER], in_=ps[:]
                    )
                nc.sync.dma_start(
                    out=out[m * 128 : (m + 1) * 128, n0 : n0 + N_OUTER], in_=o_sb[:]
                )

    nc.compile()
    _PROGRAM = nc
    return nc


def kernel(X, smp_weight):
    X = np.ascontiguousarray(np.asarray(X, dtype=np.float32))
    Wfull = np.asarray(smp_weight, dtype=np.float32)

    nc = _build()
    xt = np.ascontiguousarray(X.reshape(M, T).T)  # [100, 512]
    in_maps = [
        {"XT": xt, "W": np.ascontiguousarray(Wfull[:, i * NSH : (i + 1) * NSH])}
        for i in range(NCORES)
    ]
    res = bass_utils.run_bass_kernel_spmd(nc, in_maps, core_ids=list(range(NCORES)))
    full = np.concatenate([res.results[i]["OUT"] for i in range(NCORES)], axis=1)
    return full.reshape(B, C, N_SMP, D_PROP, T)


# revision 3
# speedup vs baseline: 1.0862x; 1.0862x over previous
"""Trainium2 Bass kernel for nn_BMSampling: out = X.reshape(B*C, T) @ smp_weight.

Strategy: tensor-parallel over the N*D*T = 320000 output columns, 8 cores x
40000 columns each. Each core runs the same program on its column shard:
  OUT_shard[512, 40000] = X^T[100, 512].T @ W_shard[100, 40000]
The kernel is output-DMA bound (~82 MB written per core), so the loop
structure streams W tiles in, keeps X^T resident in SBUF as the stationary
matmul operand, and stages 2 MB output tiles for efficient DMA out.
"""

from contextlib import ExitStack

import numpy as np

import concourse.bacc as bacc
import concourse.mybir as mybir
import concourse.tile as tile
from concourse import bass_utils

B, C, T = 4, 128, 100
N_SMP, D_PROP = 32, 100
M = B * C                     # 512 matmul rows
NDT = N_SMP * D_PROP * T      # 320000 output columns
NCORES = 8
NSH = NDT // NCORES           # 40000 columns per core

K = T                         # 100 contraction dim (on SBUF partitions)
N_OUTER = 4000                # columns per W tile / output staging tile
N_INNER = 500                 # matmul free dim (fits one PSUM bank: <=512 f32)
F32 = mybir.dt.float32

_PROGRAM = None


def _build():
    global _PROGRAM
    if _PROGRAM is not None:
        return _PROGRAM

    nc = bacc.Bacc("TRN2", debug=False)
    xt = nc.dram_tensor("XT", [K, M], F32, kind="ExternalInput").ap()
    w = nc.dram_tensor("W", [K, NSH], F32, kind="ExternalInput").ap()
    out = nc.dram_tensor("OUT", [M, NSH], F32, kind="ExternalOutput").ap()

    with tile.TileContext(nc) as tc, ExitStack() as ctx:
        xpool = ctx.enter_context(tc.tile_pool(name="x", bufs=1))
        wpool = ctx.enter_context(tc.tile_pool(name="w", bufs=4))
        opool = ctx.enter_context(tc.tile_pool(name="o", bufs=3))
        pspool = ctx.enter_context(tc.tile_pool(name="ps", bufs=4, space="PSUM"))

        # W loads ride the ACT HWDGE ring so they never queue behind the 2 MB
        # output writes on the SP ring (head-of-line blocking starves PE).
        xt_sb = xpool.tile([K, M], F32)
        nc.scalar.dma_start(out=xt_sb[:], in_=xt[:])

        for n0 in range(0, NSH, N_OUTER):
            w_sb = wpool.tile([K, N_OUTER], F32)
            nc.scalar.dma_start(out=w_sb[:], in_=w[:, n0 : n0 + N_OUTER])
            for m in range(M // 128):
                o_sb = opool.tile([128, N_OUTER], F32)
                for j in range(N_OUTER // (2 * N_INNER)):
                    ps = pspool.tile([128, 2, 512], F32)  # one PSUM bank per slot
                    for h in range(2):
                        jj = 2 * j + h
                        nc.tensor.matmul(
                            ps[:, h, :N_INNER],
                            xt_sb[:, m * 128 : (m + 1) * 128],
                            w_sb[:, jj * N_INNER : (jj + 1) * N_INNER],
                            start=True,
                            stop=True,
                        )
                    nc.vector.tensor_copy(
                        out=o_sb[
                            :, 2 * j * N_INNER : 2 * (j + 1) * N_INNER
                        ].rearrange("p (a b) -> p a b", a=2),
                        in_=ps[:, :, :N_INNER],
                    )
                nc.sync.dma_start(
                    out=out[m * 128 : (m + 1) * 128, n0 : n0 + N_OUTER], in_=o_sb[:]
                )

    nc.compile()
    _PROGRAM = nc
    return nc


def kernel(X, smp_weight):
    X = np.ascontiguousarray(np.asarray(X, dtype=np.float32))
    Wfull = np.asarray(smp_weight, dtype=np.float32)

    nc = _build()
    xt = np.ascontiguousarray(X.reshape(M, T).T)  # [100, 512]
    in_maps = [
        {"XT": xt, "W": np.ascontiguousarray(Wfull[:, i * NSH : (i + 1) * NSH])}
        for i in range(NCORES)
    ]
    res = bass_utils.run_bass_kernel_spmd(nc, in_maps, core_ids=list(range(NCORES)))
    full = np.concatenate([res.results[i]["OUT"] for i in range(NCORES)], axis=1)
    return full.reshape(B, C, N_SMP, D_PROP, T)
